# revision 8
# baseline (speedup 1.0000x reference)
"""Local (banded) attention -> mean over sequence, on 8 TRN2 NeuronCores. v2.

Same math as baseline (see kernel.py docstring) but:
- fp8 e4m3 score path: qa = x@A+cb via DoubleRow fp8 matmuls (contract 256 in
  one instruction at 0.5 cyc/row), scores likewise.
- cb bias folded into the qa matmul group as a rank-1 (contract=1) matmul so
  the psum->sbuf quantization is a pure dtype convert (split Act/DVE).
- exp in triple-block strided Act ops over 3 psum banks at once.
- u = tw @ x flipped to f=1 matmuls (lhsT=xn chunk, rhs=twT column).
- 7 big DMAs instead of 17 (HWDGE descriptor cost is 625ns each, serial).
- PE warmup matmuls during the input-DMA latency to win the p-state ramp.
"""

import numpy as np
import ml_dtypes

B, S, H = 4, 4096, 256
W = 128          # window size this kernel is specialized for
SH = S // 2      # query rows per core
HALO = 128
NK = SH + 2 * HALO   # keys per core incl. zero-padded halo (2304)
NKC = NK // 128      # 18 key chunks
NQB = SH // 128      # 16 query blocks
BF16 = ml_dtypes.bfloat16
F8 = ml_dtypes.float8_e4m3

_CACHE = {}
ASCALE = 16.0   # A is scaled up into fp8's normal range; exp un-scales

# f8 pack layout (fp8, [128, 5376]):
#   [0:512)     a8 planes: a8[p, t*256+m] = A[t*128+p, m]
#   [512:768)   cb8 row (partition 0 only)
#   [768:3072)  xT8 plane 0: x[key, hidden 0:128].T
#   [3072:5376) xT8 plane 1: x[key, hidden 128:256].T
# bh pack layout (bf16, [128, 4992]): [0:384) mk band mask, [384:4992) xn
#   chunk-major: bh[p, 384 + c*256 + d] = xpad[c*128+p, d]
F8W = 5376
BHW = 4992


def _build():
    import concourse.bass as bass
    import concourse.tile as tile
    import concourse.mybir as mybir
    from concourse import bacc

    f32 = mybir.dt.float32
    bf16 = mybir.dt.bfloat16
    fp8 = mybir.dt.float8e4
    DR = mybir.MatmulPerfMode.DoubleRow
    AF = mybir.ActivationFunctionType
    ALU = mybir.AluOpType

    nc = bacc.Bacc(
        "TRN2", target_bir_lowering=False, debug=False,
        enable_asserts=False, num_devices=1,
    )

    f8_d = nc.dram_tensor("f8", [128, F8W], fp8, kind="ExternalInput").ap()
    bh_d = nc.dram_tensor("bh", [128, BHW], bf16, kind="ExternalInput").ap()
    rc_d = nc.dram_tensor("rc", [128, NQB + 2], f32, kind="ExternalInput").ap()
    u_d = nc.dram_tensor("u", [128, 2], f32, kind="ExternalOutput").ap()

    with tile.TileContext(nc) as tc:
        with (
            tc.tile_pool(name="cst", bufs=1) as cst,
            tc.tile_pool(name="big", bufs=1) as big,
            tc.tile_pool(name="pex", bufs=3) as pex,
            tc.tile_pool(name="pem", bufs=4) as pem,
            tc.tile_pool(name="pbig", bufs=1, space="PSUM") as pbig,
            tc.tile_pool(name="ptw", bufs=1, space="PSUM") as ptw,
            tc.tile_pool(name="pu", bufs=1, space="PSUM") as pu,
        ):
            f8sb = big.tile([128, F8W], fp8, tag="f8sb")
            bhsb = big.tile([128, BHW], bf16, tag="bhsb")
            q8 = big.tile([128, 4096], fp8, tag="q8")
            rc_sb = cst.tile([128, NQB + 2], f32, tag="rc")
            rs_all = cst.tile([128, NQB], f32, tag="rs")
            iv_all = cst.tile([128, NQB], f32, tag="iv")
            ivb_all = cst.tile([128, NQB], bf16, tag="ivb")
            twT = cst.tile([128, NKC], bf16, tag="twT")
            u_sb = cst.tile([128, 2], f32, tag="usb")
            wmA = cst.tile([128, 128], bf16, tag="wmA")
            wmB = cst.tile([128, 256], bf16, tag="wmB")

            T1 = pbig.tile([128, 1536], f32, tag="T1")
            T2 = pbig.tile([128, 1536], f32, tag="T2")
            twb = ptw.tile([128, 512], f32, tag="twp")
            pub = pu.tile([128, 512], f32, tag="up")
            twp = twb[:, 0:NKC]
            up = pub[:, 0:2]

            nc.gpsimd.memset(wmA[:], 0.0)
            nc.gpsimd.memset(wmB[:], 0.0)
            dumact = cst.tile([1, 1], f32, tag="dumact")
            # tiny activation with no DMA deps: hoists LoadActFuncSet to t~0.7
            nc.scalar.activation(dumact[:], wmA[0:1, 0:1], AF.Exp)

            # input DMAs: first two cover a8+cb8 and both xT8 planes' first
            # 640 keys (enough for qa q0:512 and score blocks 0-1)
            nc.sync.dma_start(f8sb[:], f8_d[:])
            nc.sync.dma_start(rc_sb[:], rc_d[:])
            nc.sync.dma_start(bhsb[:, 0:384], bh_d[:, 0:384])
            nc.gpsimd.dma_start(bhsb[:, 384:2688], bh_d[:, 384:2688])
            nc.gpsimd.dma_start(bhsb[:, 2688:4992], bh_d[:, 2688:4992])

            a8v = f8sb[:, 0:512].rearrange("p (t m) -> p t m", t=2)
            cb8 = f8sb[0:1, 512:768]
            xT8 = f8sb[:, 768:5376].rearrange("p (t k) -> p t k", t=2)
            q8v = q8.rearrange("p (t q) -> p t q", t=2)
            mk = bhsb[:, 0:384]
            xnv = bhsb[:, 384:4992].rearrange("p (c d) -> p c d", d=256)

            # PE warmup: keep the tensor engine continuously busy through the
            # input-DMA latency so real matmuls run at full p-state.
            for w in range(5):
                nc.tensor.matmul(T2[:, 1024:1280], wmA[:], wmB[:],
                                 start=True, stop=True)

            banks = [T1[:, 0:512], T1[:, 512:1024],
                     T1[:, 1024:1536], T2[:, 0:512],
                     T2[:, 512:1024], T2[:, 1024:1536],
                     twb[:, 0:512], pub[:, 0:512]]
            fills = [(m, qw) for qw in range(4) for m in range(2)]
            ACTQ = {1, 3, 6, 7}         # quant fills done on Act (rest DVE)

            def do_fill(fi, mm_only=False):
                m, qw = fills[fi]
                bank = banks[fi]
                q0 = qw * 512
                nc.tensor.matmul(
                    bank[:, 0:256], a8v[:, :, m * 128:(m + 1) * 128],
                    xT8[:, :, HALO + q0: HALO + q0 + 256],
                    start=True, stop=False, perf_mode=DR)
                nc.tensor.matmul(
                    bank[:, 256:512], a8v[:, :, m * 128:(m + 1) * 128],
                    xT8[:, :, HALO + q0 + 256: HALO + q0 + 512],
                    start=False, stop=True, perf_mode=DR)
                if not mm_only:
                    do_quant(fi)

            def do_quant(fi):
                m, qw = fills[fi]
                q0 = qw * 512
                dst = q8v[:, m, q0:q0 + 512]
                cbm = rc_sb[:, NQB + m: NQB + m + 1]
                if fi in ACTQ:
                    nc.scalar.activation(dst, banks[fi][:], AF.Identity,
                                         bias=cbm)
                else:
                    nc.vector.tensor_scalar_add(dst, banks[fi][:], cbm)

            TRIPLES = [(0, 1, 2), (3, 4, 5), (6, 7, 8), (9, 10, 11),
                       (12, 13, 14), (15,)]
            em_at = {}

            def emit_chunk(jc):
                bs = [i for i in range(jc - 2, jc + 1) if 0 <= i < NQB]
                for n, i in enumerate(bs):
                    emt, off = em_at[i]
                    c0 = off + (jc - i) * 128
                    nc.tensor.matmul(
                        twp[:, jc:jc + 1], emt[:, c0:c0 + 128],
                        ivb_all[:, i:i + 1],
                        start=(n == 0), stop=(n == len(bs) - 1))

            for fi in range(2):
                do_fill(fi, mm_only=True)
            for w in range(6):
                nc.tensor.matmul(pub[:, 0:256], wmA[:], wmB[:],
                                 start=True, stop=True)
            for fi in range(2, 8):
                do_fill(fi, mm_only=True)
            for fi in (0, 1, 2, 3):
                do_quant(fi)
            # quants 4-7 are interleaved into the Act/DVE streams below
            QSCHED = {0: (4, 5), 1: (6, 7)}
            for g, blocks in enumerate(TRIPLES):
                T = T1 if g % 2 == 0 else T2
                nb = len(blocks)
                ex = pex.tile([128, 1152], bf16, tag="ex3", name=f"ex_{g}")
                em = pem.tile([128, 1152], bf16, tag="em3", name=f"em_{g}")
                for j, i in enumerate(blocks):
                    nc.tensor.matmul(
                        T[:, 512 * j: 512 * j + 384],
                        q8v[:, :, 128 * i: 128 * i + 128],
                        xT8[:, :, 128 * i: 128 * i + 384],
                        start=True, stop=True, perf_mode=DR)
                if g in QSCHED:
                    for fi in QSCHED[g]:
                        do_quant(fi)
                if g >= 2:
                    for jc in TRIPLES[g - 2]:
                        emit_chunk(jc)
                    if g in (3, 5):
                        c0 = 3 * (g - 3)
                        nc.vector.tensor_copy(twT[:, c0:c0 + 6],
                                              twp[:, c0:c0 + 6])
                Tv = T.rearrange("p (b c) -> p b c", c=512)
                exv = ex.rearrange("p (b c) -> p b c", c=384)
                nc.scalar.activation(exv[:, 0:nb, :], Tv[:, 0:nb, 0:384],
                                     AF.Exp, scale=1.0 / ASCALE)
                for j, i in enumerate(blocks):
                    rs0 = cst.tile([128, 1], f32, tag=f"rs0{i % 3}",
                                   name=f"rs0_{i}")
                    nc.vector.scalar_tensor_tensor(
                        em[:, 384 * j:384 * (j + 1)],
                        ex[:, 384 * j:384 * (j + 1)], 1.0, mk[:],
                        ALU.mult, ALU.mult, accum_out=rs0[:])
                    nc.vector.tensor_scalar_add(
                        rs_all[:, i:i + 1], rs0[:], rc_sb[:, i:i + 1])
                    em_at[i] = (em, 384 * j)
                if g in (1, 3):
                    gs = slice(TRIPLES[g - 1][0], blocks[-1] + 1)
                elif g in (4, 5):
                    gs = slice(blocks[0], blocks[-1] + 1)
                else:
                    gs = None
                if gs is not None:
                    nc.vector.reciprocal(iv_all[:, gs], rs_all[:, gs])
                    nc.vector.tensor_copy(ivb_all[:, gs], iv_all[:, gs])

            for jc in (12, 13, 14, 15, 16, 17):
                emit_chunk(jc)
            nc.vector.tensor_copy(twT[:, 12:18], twp[:, 12:18])
            for j2 in range(NKC):
                for hh in range(2):
                    nc.tensor.matmul(
                        up[:, hh:hh + 1],
                        xnv[:, j2, hh * 128:(hh + 1) * 128],
                        twT[:, j2:j2 + 1],
                        start=(j2 == 0 and hh == 0),
                        stop=(j2 == NKC - 1 and hh == 1))
            nc.vector.tensor_copy(u_sb[:], up[:])
            nc.sync.dma_start(u_d[:], u_sb[:])

    nc.compile()
    return nc


def _numpy_fallback(x, Wq, bq, Wk, bk, Wv, bv, window_size):
    out = np.zeros((B, H), np.float64)
    xs = x.astype(np.float64)
    A = (Wq.astype(np.float64) @ Wk.astype(np.float64).T) / np.sqrt(H)
    cb = (Wk.astype(np.float64) @ bq.astype(np.float64)) / np.sqrt(H)
    idx = np.arange(x.shape[1])
    band = np.abs(idx[:, None] - idx[None, :]) <= int(window_size)
    for b in range(x.shape[0]):
        qa = xs[b] @ A + cb
        sc = qa @ xs[b].T
        e = np.exp(sc - sc.max(axis=-1, keepdims=True)) * band
        w = e / e.sum(-1, keepdims=True)
        tw = w.sum(axis=0)
        out[b] = (tw @ xs[b] / x.shape[1]) @ Wv.astype(np.float64) + bv
    return out.astype(np.float32)


def kernel(x, Wq, bq, Wk, bk, Wv, bv, window_size):
    x = np.asarray(x)
    Wq, bq = np.asarray(Wq), np.asarray(bq)
    Wk, bk = np.asarray(Wk), np.asarray(bk)
    Wv, bv = np.asarray(Wv), np.asarray(bv)
    if int(window_size) != W or x.shape != (B, S, H):
        return _numpy_fallback(x, Wq, bq, Wk, bk, Wv, bv, window_size)

    from concourse.bass_utils import run_bass_kernel_spmd

    if "nc" not in _CACHE:
        _CACHE["nc"] = _build()
    nc = _CACHE["nc"]

    A64 = (Wq.astype(np.float64) @ Wk.astype(np.float64).T) / np.sqrt(H)
    cb64 = (Wk.astype(np.float64) @ bq.astype(np.float64)) / np.sqrt(H)
    a8 = ((A64 * ASCALE).astype(F8).reshape(2, 128, 256)
          .transpose(1, 0, 2).reshape(128, 512))
    r = np.arange(128)[:, None]
    c = np.arange(384)[None, :]
    mk_np = (np.abs(c - r - HALO) <= W).astype(BF16)

    in_maps = []
    for core in range(8):
        b, h = core // 2, core % 2
        q0 = h * SH
        xpad = np.zeros((NK, H), np.float32)
        lo, hi = q0 - HALO, q0 + SH + HALO
        slo, shi = max(lo, 0), min(hi, S)
        xpad[slo - lo: shi - lo, :] = x[b, slo:shi, :]
        xT8 = np.ascontiguousarray(xpad.T).astype(F8)   # [256, 2304]
        f8p = np.zeros((128, F8W), F8)
        f8p[:, 0:512] = a8
        f8p[0, 512:768] = (cb64 * ASCALE).astype(F8)
        f8p[:, 768:3072] = xT8[0:128]
        f8p[:, 3072:5376] = xT8[128:256]
        bhp = np.zeros((128, BHW), BF16)
        bhp[:, 0:384] = mk_np
        bhp[:, 384:] = (xpad.astype(BF16).reshape(NKC, 128, 256)
                        .transpose(1, 0, 2).reshape(128, NKC * 256))
        rc_np = np.zeros((128, NQB + 2), np.float32)
        rc_np[:, NQB:NQB + 2] = (cb64 * ASCALE).astype(np.float32).reshape(2, 128).T
        rr = np.arange(128)
        if h == 0:
            rc_np[:, 0] = -(128 - rr).astype(np.float32)
        else:
            rc_np[:, NQB - 1] = -(rr + 1).astype(np.float32)
        in_maps.append({"f8": f8p, "bh": bhp, "rc": rc_np})

    import os
    trace = bool(os.environ.get("BASS_TRACE"))
    res = run_bass_kernel_spmd(nc, in_maps, list(range(8)), trace=trace)
    _CACHE["last"] = res

    out = np.zeros((B, H), np.float64)
    for b in range(B):
        u = np.zeros(256, np.float64)
        for h in range(2):
            uc = res.results[2 * b + h]["u"].astype(np.float64)
            u += np.concatenate([uc[:, 0], uc[:, 1]])
        out[b] = (u / S) @ Wv.astype(np.float64) + bv
    return out.astype(np.float32)
